# revision 2
# baseline (speedup 1.0000x reference)
"""Trainium2 Bass kernel: scaled-softmax attention, B=4 H=16 S=2048 D=64.

Sharding: batch*heads (64) across 8 NeuronCores, 8 heads per core.

Per head, on-device (flash-style streaming over k-blocks):
  for each k-block kb (128 keys):
    W[kb] = kT_aug[kb] @ qT_aug   (fp16 matmuls, contraction 65 = 64 dims
            + fused row; produces the PRE-SCALED logit wave
            t = A*(s - m_hat) with A = 128/ln2, so both exp paths below
            read it directly)
    P[kb] split between two engines (both write the same bf16 tile):
      cols [0:QA):    ACT exp with scale=1/A  ->  exp(s - m_hat)  (exact)
      cols [QA:1024): DVE tensor_scalar (t + 16250) max 0 -> int16, whose
                      bits reinterpreted as bf16 are Schraudolph's
                      approximate exp (rel err ~ +-3%, linear-in-mantissa)
    av[qc] += [v|1][kb] @ P[kb]   (fp16 x bf16, K=128 accumulated in PSUM;
              the ones-column makes row 64 the softmax denominator)
  outT (rows 0..63 = unnormalized out^T, row 64 = denominator) -> HBM.

fp16 for qT/kT keeps 11-bit-mantissa precision (same class as fp32r) but
runs matmuls at the full 1 col/cycle rate with cheap shadow weight loads
(fp32r reloads 4-byte weights per matmul and measures ~2x slower).

Host (numpy) does input/output marshaling: q scaled by A/(scale_factor*
inv_scale), m_hat = 5*||qs||, transpose/augment/fp16 rounding on the way
in; per-query divide by the denominator row + transpose on the way out.
Softmax normalization cancels any per-query common factor, so m_hat only
needs to keep exp in range, and the fp16 rounding of the m_hat row is
harmless.
"""

import os
import sys

sys.path.insert(0, "/opt/trn_rl_repo")

from contextlib import ExitStack

import numpy as np

import concourse.bass as bass
import concourse.tile as tile
from concourse import bacc, mybir
from concourse.bass_utils import run_bass_kernel_spmd
from concourse.masks import make_identity

B, H, S, D = 4, 16, 2048, 64
N_CORES = 8
HPC = (B * H) // N_CORES  # heads per core
KB = S // 128  # 16 k-blocks
QC = S // 512  # 4 q-chunks
DA = D + 1  # augmented contraction dim (65)

A_SCALE = np.float32(128.0 / np.log(2.0))  # Schraudolph pre-scale
TS_BIAS = 16250.0  # 127<<7 bias, less 6.0 tuned to center the sawtooth err
QA = 640  # queries per 1024-wave handled by ACT (rest on DVE)

F32 = mybir.dt.float32
BF16 = mybir.dt.bfloat16
F16 = mybir.dt.float16
I16 = mybir.dt.int16

LAST_RESULT = None
_CACHED_NC = None


def _maybe_install_ntff_hook():
    """BASS_TRACE=1 needs antenv.axon_hooks, absent from this image; inject it."""
    if not os.environ.get("BASS_TRACE") or "antenv.axon_hooks" in sys.modules:
        return
    try:
        import types

        import antenv
        from trn_agent_boot.trn_boot import _ntff_profile_via_ctypes

        mod = types.ModuleType("antenv.axon_hooks")
        mod._hook = None
        mod.set_axon_ntff_profile_hook = lambda h: setattr(mod, "_hook", h)
        mod.get_axon_ntff_profile_hook = lambda: mod._hook
        sys.modules["antenv.axon_hooks"] = mod
        antenv.axon_hooks = mod
        mod.set_axon_ntff_profile_hook(
            _ntff_profile_via_ctypes("/opt/axon/libaxon_pjrt.so")
        )
    except Exception:
        os.environ["BASS_NEVER_TRACE"] = "1"


def _build_nc():
    nc = bacc.Bacc("TRN2", target_bir_lowering=False, debug=False)

    d_qT = nc.dram_tensor("qT", [HPC, DA, S], F16, kind="ExternalInput").ap()
    d_kT = nc.dram_tensor("kT", [HPC, DA, S], F16, kind="ExternalInput").ap()
    d_v = nc.dram_tensor("v", [HPC, 128, KB, DA], F16, kind="ExternalInput").ap()
    d_out = nc.dram_tensor("outT", [HPC, DA, S], F32, kind="ExternalOutput").ap()

    with tile.TileContext(nc) as tc, ExitStack() as ctx:
        cpool = ctx.enter_context(tc.tile_pool(name="consts", bufs=1))
        inpool = ctx.enter_context(tc.tile_pool(name="in", bufs=3))
        ptpool = ctx.enter_context(tc.tile_pool(name="pt", bufs=12))
        wkpool = ctx.enter_context(tc.tile_pool(name="wk", bufs=3))
        qkp = ctx.enter_context(tc.tile_pool(name="qkp", bufs=2, space="PSUM"))
        mp = ctx.enter_context(tc.tile_pool(name="mp", bufs=1, space="PSUM"))

        ident = cpool.tile([DA, DA], F32)
        make_identity(nc, ident[:])
        t_warm = cpool.tile([1, 1], F32)
        # trigger the ACT exp table load while input DMAs run
        nc.scalar.activation(
            t_warm[:], ident[0:1, 0:1], mybir.ActivationFunctionType.Exp
        )

        inv_a = float(1.0 / A_SCALE)

        for h in range(HPC):
            t_qT = inpool.tile([DA, S], F16, tag="qT")
            t_kT = inpool.tile([DA, S], F16, tag="kT")
            t_v = inpool.tile([128, KB, DA], F16, tag="v")
            nc.sync.dma_start(out=t_kT[:, 0:256], in_=d_kT[h][:, 0:256])
            nc.sync.dma_start(out=t_qT[:, 0:1024], in_=d_qT[h][:, 0:1024])
            nc.sync.dma_start(out=t_qT[:, 1024:2048], in_=d_qT[h][:, 1024:2048])
            nc.sync.dma_start(out=t_kT[:, 256:2048], in_=d_kT[h][:, 256:2048])
            nc.sync.dma_start(out=t_v[:], in_=d_v[h])

            p_av = [
                mp.tile([DA, 512], F32, tag=f"av{qc}", name=f"av{qc}_{h}")
                for qc in range(QC)
            ]

            pts = []
            for kb in range(KB):
                t_pt = ptpool.tile([128, S], BF16, tag="pt", name=f"pt{h}_{kb}")
                t_pt_i16 = t_pt[:].bitcast(I16)
                pts.append(t_pt)
                for qh in range(2):
                    pw = qkp.tile([128, 1024], F32, tag="wave")
                    for j in range(2):
                        qc = qh * 2 + j
                        nc.tensor.matmul(
                            pw[:, j * 512 : (j + 1) * 512],
                            t_kT[:, kb * 128 : (kb + 1) * 128],
                            t_qT[:, qc * 512 : (qc + 1) * 512],
                            start=True,
                            stop=True,
                        )
                    # exact exp on ACT for the first QA queries of the wave
                    nc.scalar.activation(
                        t_pt[:, qh * 1024 : qh * 1024 + QA],
                        pw[:, 0:QA],
                        mybir.ActivationFunctionType.Exp,
                        bias=0.0,
                        scale=inv_a,
                    )
                    # Schraudolph exp on DVE for the rest: bf16 bits via i16
                    nc.vector.tensor_scalar(
                        out=t_pt_i16[:, qh * 1024 + QA : (qh + 1) * 1024],
                        in0=pw[:, QA:1024],
                        scalar1=TS_BIAS,
                        scalar2=0.0,
                        op0=mybir.AluOpType.add,
                        op1=mybir.AluOpType.max,
                    )
            for kb in range(KB):
                for qc in range(QC):
                    nc.tensor.matmul(
                        p_av[qc][:],
                        t_v[:, kb, :],
                        pts[kb][:, qc * 512 : (qc + 1) * 512],
                        start=(kb == 0),
                        stop=(kb == KB - 1),
                    )

            # drain accumulators: outT rows 0..63 = unnormalized out^T,
            # row 64 = denominator; host divides + transposes
            t_outT = wkpool.tile([DA, S], F32, tag="outT")
            for qc in range(QC):
                nc.vector.tensor_copy(
                    t_outT[:, qc * 512 : (qc + 1) * 512], p_av[qc][:]
                )
            nc.sync.dma_start(out=d_out[h], in_=t_outT[:])

    nc.compile()
    return nc


def kernel(
    q: np.ndarray,
    k: np.ndarray,
    v: np.ndarray,
    scale_factor: np.ndarray,
    inv_scale: np.ndarray,
) -> np.ndarray:
    global LAST_RESULT, _CACHED_NC

    q = np.asarray(q, np.float32)
    k = np.asarray(k, np.float32)
    v = np.asarray(v, np.float32)
    scale_factor = np.asarray(scale_factor, np.float32)
    inv_scale = np.asarray(inv_scale, np.float32)

    # host-side input marshaling
    r = 1.0 / (scale_factor * inv_scale[..., None])  # [B,H,S]
    qs = q * r[..., None]  # [B,H,S,D]
    mhat = 5.0 * np.sqrt((qs.astype(np.float64) ** 2).sum(-1)).astype(np.float32)
    q_aug = np.concatenate(
        [A_SCALE * qs, (-A_SCALE) * mhat[..., None]], axis=-1
    )  # [B,H,S,DA] pre-scaled for Schraudolph
    k_aug = np.concatenate([k, np.ones((B, H, S, 1), np.float32)], axis=-1)
    v_aug = np.concatenate([v, np.ones((B, H, S, 1), np.float32)], axis=-1)

    qT = np.ascontiguousarray(q_aug.transpose(0, 1, 3, 2)).astype(np.float16)
    kT = np.ascontiguousarray(k_aug.transpose(0, 1, 3, 2)).astype(np.float16)
    # [B,H,S,DA] -> [B,H,KB,128,DA] -> [B,H,128,KB,DA]
    v16 = np.ascontiguousarray(
        v_aug.reshape(B, H, KB, 128, DA).transpose(0, 1, 3, 2, 4)
    ).astype(np.float16)

    qT = qT.reshape(N_CORES, HPC, DA, S)
    kT = kT.reshape(N_CORES, HPC, DA, S)
    v16 = v16.reshape(N_CORES, HPC, 128, KB, DA)

    _maybe_install_ntff_hook()
    if _CACHED_NC is None:
        _CACHED_NC = _build_nc()
    nc = _CACHED_NC

    in_maps = [{"qT": qT[c], "kT": kT[c], "v": v16[c]} for c in range(N_CORES)]
    res = run_bass_kernel_spmd(nc, in_maps, list(range(N_CORES)))
    LAST_RESULT = res
    outT = np.stack([res.results[c]["outT"] for c in range(N_CORES)])  # [8,HPC,DA,S]
    out = outT[:, :, :D, :] / outT[:, :, D : D + 1, :]
    return (
        np.ascontiguousarray(out.transpose(0, 1, 3, 2))
        .reshape(B, H, S, D)
        .astype(np.float32)
    )


# revision 6
# speedup vs baseline: 1.1121x; 1.1121x over previous
"""Trainium2 Bass kernel: scaled-softmax attention, B=4 H=16 S=2048 D=64.

Sharding: batch*heads (64) across 8 NeuronCores, 8 heads per core.

Per head, on-device (flash-style streaming over k-blocks):
  for each k-block kb (128 keys):
    W[kb] = kT_aug[kb] @ qT_aug   (fp16 matmuls, contraction 65 = 64 dims
            + fused row; produces the PRE-SCALED logit wave
            t = A*(s - m_hat) with A = 128/ln2, so both exp paths below
            read it directly)
    P[kb] split between two engines (both write the same bf16 tile):
      cols [0:QA):    ACT exp with scale=1/A  ->  exp(s - m_hat)  (exact)
      cols [QA:1024): DVE tensor_scalar (t + 16250) max 0 -> int16, whose
                      bits reinterpreted as bf16 are Schraudolph's
                      approximate exp (rel err ~ +-3%, linear-in-mantissa)
    av[qc] += [v|1][kb] @ P[kb]   (fp16 x bf16, K=128 accumulated in PSUM;
              the ones-column makes row 64 the softmax denominator)
  outT (rows 0..63 = unnormalized out^T, row 64 = denominator) -> HBM.

fp16 for qT/kT keeps 11-bit-mantissa precision (same class as fp32r) but
runs matmuls at the full 1 col/cycle rate with cheap shadow weight loads
(fp32r reloads 4-byte weights per matmul and measures ~2x slower).

Host (numpy) does input/output marshaling: q scaled by A/(scale_factor*
inv_scale), m_hat = 5*||qs||, transpose/augment/fp16 rounding on the way
in; per-query divide by the denominator row + transpose on the way out.
Softmax normalization cancels any per-query common factor, so m_hat only
needs to keep exp in range, and the fp16 rounding of the m_hat row is
harmless.
"""

import os
import sys

sys.path.insert(0, "/opt/trn_rl_repo")

from contextlib import ExitStack

import numpy as np

import concourse.bass as bass
import concourse.tile as tile
from concourse import bacc, mybir
from concourse.bass_utils import run_bass_kernel_spmd
from concourse.masks import make_identity

B, H, S, D = 4, 16, 2048, 64
N_CORES = 8
HPC = (B * H) // N_CORES  # heads per core
KB = S // 128  # 16 k-blocks
QC = S // 512  # 4 q-chunks
DA = D + 1  # augmented contraction dim (65)

A_SCALE = np.float32(128.0 / np.log(2.0))  # Schraudolph pre-scale
TS_BIAS = 16250.0  # 127<<7 bias, less 6.0 tuned to center the sawtooth err
# Each 1024-query wave goes entirely to ONE exp engine, in a period-3
# pattern (ACT, ACT, DVE): whole-wave assignment keeps every AV matmul
# dependent on exactly one exp instruction (a mid-wave split would stall
# the PE on the slower engine), and 1/3 on DVE keeps the Schraudolph
# approximation's contribution to the output error small.
WAVE_PATTERN = ("A", "A", "D")

F32 = mybir.dt.float32
BF16 = mybir.dt.bfloat16
F16 = mybir.dt.float16
I16 = mybir.dt.int16

LAST_RESULT = None
_CACHED_NC = None


def _maybe_install_ntff_hook():
    """BASS_TRACE=1 needs antenv.axon_hooks, absent from this image; inject it."""
    if not os.environ.get("BASS_TRACE") or "antenv.axon_hooks" in sys.modules:
        return
    try:
        import types

        import antenv
        from trn_agent_boot.trn_boot import _ntff_profile_via_ctypes

        mod = types.ModuleType("antenv.axon_hooks")
        mod._hook = None
        mod.set_axon_ntff_profile_hook = lambda h: setattr(mod, "_hook", h)
        mod.get_axon_ntff_profile_hook = lambda: mod._hook
        sys.modules["antenv.axon_hooks"] = mod
        antenv.axon_hooks = mod
        mod.set_axon_ntff_profile_hook(
            _ntff_profile_via_ctypes("/opt/axon/libaxon_pjrt.so")
        )
    except Exception:
        os.environ["BASS_NEVER_TRACE"] = "1"


def _build_nc():
    nc = bacc.Bacc("TRN2", target_bir_lowering=False, debug=False)

    d_qT = nc.dram_tensor("qT", [HPC, DA, S], F16, kind="ExternalInput").ap()
    d_kT = nc.dram_tensor("kT", [HPC, DA, S], F16, kind="ExternalInput").ap()
    d_v = nc.dram_tensor("v", [HPC, 128, KB, DA], F16, kind="ExternalInput").ap()
    d_out = nc.dram_tensor("outT", [HPC, DA, S], F32, kind="ExternalOutput").ap()

    with tile.TileContext(nc) as tc, ExitStack() as ctx:
        cpool = ctx.enter_context(tc.tile_pool(name="consts", bufs=1))
        inpool = ctx.enter_context(tc.tile_pool(name="in", bufs=3))
        ptpool = ctx.enter_context(tc.tile_pool(name="pt", bufs=12))
        wkpool = ctx.enter_context(tc.tile_pool(name="wk", bufs=3))
        qkp = ctx.enter_context(tc.tile_pool(name="qkp", bufs=2, space="PSUM"))
        mp = ctx.enter_context(tc.tile_pool(name="mp", bufs=1, space="PSUM"))

        ident = cpool.tile([DA, DA], F32)
        make_identity(nc, ident[:])
        t_warm = cpool.tile([1, 1], F32)
        # trigger the ACT exp table load while input DMAs run
        nc.scalar.activation(
            t_warm[:], ident[0:1, 0:1], mybir.ActivationFunctionType.Exp
        )

        inv_a = float(1.0 / A_SCALE)

        for h in range(HPC):
            t_qT = inpool.tile([DA, S], F16, tag="qT")
            t_kT = inpool.tile([DA, S], F16, tag="kT")
            t_v = inpool.tile([128, KB, DA], F16, tag="v")
            nc.sync.dma_start(out=t_kT[:, 0:256], in_=d_kT[h][:, 0:256])
            nc.sync.dma_start(out=t_qT[:, 0:1024], in_=d_qT[h][:, 0:1024])
            nc.sync.dma_start(out=t_qT[:, 1024:2048], in_=d_qT[h][:, 1024:2048])
            nc.sync.dma_start(out=t_kT[:, 256:2048], in_=d_kT[h][:, 256:2048])
            nc.sync.dma_start(out=t_v[:], in_=d_v[h])

            p_av = [
                mp.tile([DA, 512], F32, tag=f"av{qc}", name=f"av{qc}_{h}")
                for qc in range(QC)
            ]

            pts = []
            for kb in range(KB):
                t_pt = ptpool.tile([128, S], BF16, tag="pt", name=f"pt{h}_{kb}")
                t_pt_i16 = t_pt[:].bitcast(I16)
                pts.append(t_pt)
                for qh in range(2):
                    pw = qkp.tile([128, 1024], F32, tag="wave")
                    for j in range(2):
                        qc = qh * 2 + j
                        nc.tensor.matmul(
                            pw[:, j * 512 : (j + 1) * 512],
                            t_kT[:, kb * 128 : (kb + 1) * 128],
                            t_qT[:, qc * 512 : (qc + 1) * 512],
                            start=True,
                            stop=True,
                        )
                    wav = 2 * kb + qh
                    eng = WAVE_PATTERN[wav % len(WAVE_PATTERN)]
                    if eng == "A":
                        # exact exp on ACT
                        nc.scalar.activation(
                            t_pt[:, qh * 1024 : (qh + 1) * 1024],
                            pw[:],
                            mybir.ActivationFunctionType.Exp,
                            bias=0.0,
                            scale=inv_a,
                        )
                    else:
                        # Schraudolph exp on DVE: bf16 bits via i16
                        nc.vector.tensor_scalar(
                            out=t_pt_i16[:, qh * 1024 : (qh + 1) * 1024],
                            in0=pw[:],
                            scalar1=TS_BIAS,
                            scalar2=0.0,
                            op0=mybir.AluOpType.add,
                            op1=mybir.AluOpType.max,
                        )
            for kb in range(KB):
                for qc in range(QC):
                    nc.tensor.matmul(
                        p_av[qc][:],
                        t_v[:, kb, :],
                        pts[kb][:, qc * 512 : (qc + 1) * 512],
                        start=(kb == 0),
                        stop=(kb == KB - 1),
                    )

            # drain accumulators: outT rows 0..63 = unnormalized out^T,
            # row 64 = denominator; host divides + transposes
            t_outT = wkpool.tile([DA, S], F32, tag="outT")
            for qc in range(QC):
                nc.vector.tensor_copy(
                    t_outT[:, qc * 512 : (qc + 1) * 512], p_av[qc][:]
                )
            nc.sync.dma_start(out=d_out[h], in_=t_outT[:])

    nc.compile()
    return nc


def kernel(
    q: np.ndarray,
    k: np.ndarray,
    v: np.ndarray,
    scale_factor: np.ndarray,
    inv_scale: np.ndarray,
) -> np.ndarray:
    global LAST_RESULT, _CACHED_NC

    q = np.asarray(q, np.float32)
    k = np.asarray(k, np.float32)
    v = np.asarray(v, np.float32)
    scale_factor = np.asarray(scale_factor, np.float32)
    inv_scale = np.asarray(inv_scale, np.float32)

    # host-side input marshaling
    r = 1.0 / (scale_factor * inv_scale[..., None])  # [B,H,S]
    qs = q * r[..., None]  # [B,H,S,D]
    mhat = 5.0 * np.sqrt((qs.astype(np.float64) ** 2).sum(-1)).astype(np.float32)
    q_aug = np.concatenate(
        [A_SCALE * qs, (-A_SCALE) * mhat[..., None]], axis=-1
    )  # [B,H,S,DA] pre-scaled for Schraudolph
    k_aug = np.concatenate([k, np.ones((B, H, S, 1), np.float32)], axis=-1)
    v_aug = np.concatenate([v, np.ones((B, H, S, 1), np.float32)], axis=-1)

    qT = np.ascontiguousarray(q_aug.transpose(0, 1, 3, 2)).astype(np.float16)
    kT = np.ascontiguousarray(k_aug.transpose(0, 1, 3, 2)).astype(np.float16)
    # [B,H,S,DA] -> [B,H,KB,128,DA] -> [B,H,128,KB,DA]
    v16 = np.ascontiguousarray(
        v_aug.reshape(B, H, KB, 128, DA).transpose(0, 1, 3, 2, 4)
    ).astype(np.float16)

    qT = qT.reshape(N_CORES, HPC, DA, S)
    kT = kT.reshape(N_CORES, HPC, DA, S)
    v16 = v16.reshape(N_CORES, HPC, 128, KB, DA)

    _maybe_install_ntff_hook()
    if _CACHED_NC is None:
        _CACHED_NC = _build_nc()
    nc = _CACHED_NC

    in_maps = [{"qT": qT[c], "kT": kT[c], "v": v16[c]} for c in range(N_CORES)]
    res = run_bass_kernel_spmd(nc, in_maps, list(range(N_CORES)))
    LAST_RESULT = res
    outT = np.stack([res.results[c]["outT"] for c in range(N_CORES)])  # [8,HPC,DA,S]
    out = outT[:, :, :D, :] / outT[:, :, D : D + 1, :]
    return (
        np.ascontiguousarray(out.transpose(0, 1, 3, 2))
        .reshape(B, H, S, D)
        .astype(np.float32)
    )
